# revision 1
# baseline (speedup 1.0000x reference)
"""nn_CPN_67740224192953 kernel: backbone conv + 7x7 head convs on 8 trn2 cores.

Device (8 cores, 2 per image = half-image each):
  - backbone 3x3 conv (K=27 im2col matmul, fp32) + relu (ACT)
  - head convs for [d=s1-s0, ref_x, ref_y] via taps-as-M matmuls:
    P[(c,tap), pos] = sum_cin W[c,cin,tap] * f[cin, pos]  (M=147, K=64, fp32);
    partials dumped non-overlapping (16 rows/slab + 6-row tail)
Host: shift-sum of tap partials (49 adds), softmax ordering + top-k,
  loc/fourier head at 512 detections (patch matmul), fourier contour
  synthesis, 4 iterations of refinement gathers (mirrors reference).
"""

import numpy as np

LAST_EXEC_NS = None
LAST_DEVICE_S = None

B, C_IN, H, W = 4, 3, 512, 512
C = 64
ORDER = 5
SAMPLES = 32
N_DET = 512
ITERS = 4
MARGIN = 3.0
K7 = 7
WP = W + 6            # padded row width 518
HALF = H // 2         # 256 rows per core
SLAB = 16             # output rows per slab
NSLAB = HALF // SLAB  # 16 slabs
FROWS = SLAB + 6      # f rows needed per slab (halo 3 top+bottom)
NF = FROWS * WP       # 11396 positions per slab
NCHUNK = (NF + 511) // 512  # 23 matmul chunks


def _build_device_program():
    import concourse.bacc as bacc
    import concourse.mybir as mybir
    from concourse.tile import TileContext

    nc = bacc.Bacc("TRN2", target_bir_lowering=False, num_devices=8)
    f32 = mybir.dt.float32
    f32r = mybir.dt.float32r
    imc_d = nc.dram_tensor("imc", [NSLAB * 27, NF], f32, kind="ExternalInput")
    wbb_d = nc.dram_tensor("wbb", [27, C], f32, kind="ExternalInput")
    w147a_d = nc.dram_tensor("w147a", [C, 128], f32, kind="ExternalInput")
    w147b_d = nc.dram_tensor("w147b", [C, 19], f32, kind="ExternalInput")
    ND = SLAB * WP
    plo_d = nc.dram_tensor("plo", [NSLAB * 128, ND], f32, kind="ExternalOutput")
    phi_d = nc.dram_tensor("phi", [NSLAB * 19, ND], f32, kind="ExternalOutput")
    plt_d = nc.dram_tensor("plt", [128, NF - ND], f32, kind="ExternalOutput")
    pht_d = nc.dram_tensor("pht", [19, NF - ND], f32, kind="ExternalOutput")

    with (
        TileContext(nc) as tc,
        tc.tile_pool(name="wpool", bufs=1) as wpool,
        tc.tile_pool(name="sb", bufs=1) as sb,
        tc.tile_pool(name="sbo", bufs=1) as sbo,
        tc.tile_pool(name="ps", bufs=2, space="PSUM") as ps,
        tc.tile_pool(name="ps3", bufs=3, space="PSUM") as ps3,
    ):
        # weights: DMA in, then re-copy on DVE so every matmul's weight dep
        # is a DVE semaphore (keeps per-matmul sync-wait count at the limit)
        wbb_r = wpool.tile([27, C], f32, tag="wbbr")
        w147a_r = wpool.tile([C, 128], f32, tag="war")
        w147b_r = wpool.tile([C, 19], f32, tag="wbr")
        nc.sync.dma_start(out=wbb_r[:], in_=wbb_d[:, :])
        nc.sync.dma_start(out=w147a_r[:], in_=w147a_d[:, :])
        nc.sync.dma_start(out=w147b_r[:], in_=w147b_d[:, :])
        wbb_t = wpool.tile([27, C], f32, tag="wbb")
        w147a_t = wpool.tile([C, 128], f32, tag="wa")
        w147b_t = wpool.tile([C, 19], f32, tag="wb")
        nc.vector.tensor_copy(wbb_t[:], wbb_r[:])
        nc.vector.tensor_copy(w147a_t[:], w147a_r[:])
        nc.vector.tensor_copy(w147b_t[:], w147b_r[:])

        for s in range(NSLAB):
            imc_t = sb.tile([27, NF], f32, tag="imc")
            f_t = sbo.tile([C, NF], f32, tag="f")
            nc.sync.dma_start(out=imc_t[:], in_=imc_d[s * 27:(s + 1) * 27, :])
            # backbone: f = relu(w27.T @ imc), relu on DVE
            for k in range(NCHUNK):
                a, b = k * 512, min((k + 1) * 512, NF)
                pbb = ps.tile([C, 512], f32, tag="pbb")
                nc.tensor.matmul(out=pbb[:, :b - a], lhsT=wbb_t[:],
                                 rhs=imc_t[:, a:b], start=True, stop=True)
                nc.scalar.activation(f_t[:, a:b], pbb[:, :b - a],
                                     mybir.ActivationFunctionType.Relu)
            # stage A: P[(c,tap), pos]
            plo_t = sbo.tile([128, NF], f32, tag="plo")
            phi_t = sbo.tile([19, NF], f32, tag="phi")
            for k in range(NCHUNK):
                a, b = k * 512, min((k + 1) * 512, NF)
                pa = ps3.tile([128, 512], f32, tag="pa")
                pb = ps3.tile([19, 512], f32, tag="pb")
                nc.tensor.matmul(out=pa[:, :b - a], lhsT=w147a_t[:],
                                 rhs=f_t[:, a:b], start=True, stop=True)
                nc.tensor.matmul(out=pb[:, :b - a], lhsT=w147b_t[:],
                                 rhs=f_t[:, a:b], start=True, stop=True)
                nc.vector.tensor_copy(plo_t[:, a:b], pa[:, :b - a])
                nc.scalar.copy(phi_t[:, a:b], pb[:, :b - a])
            nc.sync.dma_start(out=plo_d[s * 128:(s + 1) * 128, :], in_=plo_t[:, :ND])
            nc.sync.dma_start(out=phi_d[s * 19:(s + 1) * 19, :], in_=phi_t[:, :ND])
            if s == NSLAB - 1:
                nc.sync.dma_start(out=plt_d[:, :], in_=plo_t[:, ND:])
                nc.sync.dma_start(out=pht_d[:, :], in_=phi_t[:, ND:])
    nc.finalize()
    return nc


def _host_im2col(x):
    """Per (image, half): [NSLAB*27, NF] fp32 stacks; also return xg canvases."""
    out = {}
    for b in range(B):
        xg = np.zeros((C_IN, H + 8, W + 8), np.float32)
        xg[:, 4:4 + H, 4:4 + W] = x[b]
        sw = np.lib.stride_tricks.sliding_window_view(xg, (3, 3), axis=(1, 2))
        # sw[c, i, j, dy, dx] = xg[c, i+dy, j+dx]
        for h in range(2):
            base0 = h * HALF
            cols = []
            for s in range(NSLAB):
                r0 = base0 + s * SLAB - 3  # image row of f-row j=0
                # f(R, q): need sw[c, R+3, q, dy, dx]  (R=r0+j, q in [0,518))
                blk = sw[:, r0 + 3:r0 + 3 + FROWS, 0:WP, :, :]
                imc = np.ascontiguousarray(
                    blk.transpose(0, 3, 4, 1, 2)).reshape(27, FROWS, WP)
                # zero f positions that must be conv-padding zeros
                imc[:, :, 0:3] = 0.0
                imc[:, :, WP - 3:WP] = 0.0
                rows = r0 + np.arange(FROWS)
                bad = (rows < 0) | (rows >= H)
                if bad.any():
                    imc[:, bad, :] = 0.0
                cols.append(imc.reshape(27, NF))
            out[(b, h)] = np.concatenate(cols, 0)
    return out


def _shift_sum(rr):
    """Non-overlap dumps -> maps [3, HALF, WP] for one core.

    Slab s dumps P rows [0,16) (= global f-rows S0-3..S0+13); the last
    slab's rows [16,22) arrive via the tail tensors. Global P covers
    f-rows [-3, HALF+3)."""
    plo = rr["plo"].reshape(NSLAB, 128, SLAB, WP)
    phi = rr["phi"].reshape(NSLAB, 19, SLAB, WP)
    G = np.empty((147, HALF + 6, WP), np.float32)
    for s in range(NSLAB):
        G[:128, s * SLAB:(s + 1) * SLAB] = plo[s]
        G[128:, s * SLAB:(s + 1) * SLAB] = phi[s]
    G[:128, HALF:] = rr["plt"].reshape(128, 6, WP)
    G[128:, HALF:] = rr["pht"].reshape(19, 6, WP)
    out = np.zeros((3, HALF, WP), np.float32)
    for c in range(3):
        for dy in range(K7):
            for dx in range(K7):
                m = c * 49 + dy * K7 + dx
                srcv = G[m, dy:dy + HALF, :]
                sh = dx - 3
                if sh == 0:
                    out[c] += srcv
                elif sh > 0:
                    out[c, :, :WP - sh] += srcv[:, sh:]
                else:
                    out[c, :, -sh:] += srcv[:, :WP + sh]
    return out


def kernel(x, w_bb, b_bb, w_score, b_score, w_loc, b_loc,
           w_fourier, b_fourier, w_ref, b_ref):
    x = np.asarray(x, np.float32)
    w_bb = np.asarray(w_bb, np.float32)
    w_score = np.asarray(w_score, np.float32)
    w_loc = np.asarray(w_loc, np.float32)
    w_fourier = np.asarray(w_fourier, np.float32)
    w_ref = np.asarray(w_ref, np.float32)
    b_bb = np.asarray(b_bb, np.float32)

    # ---- weights prep ----
    w27 = np.ascontiguousarray(w_bb.transpose(1, 2, 3, 0).reshape(27, C))
    w_d = (w_score[1] - w_score[0]).astype(np.float32)          # [C,7,7]
    whead = np.stack([w_d, w_ref[0], w_ref[1]], 0)              # [3,C,7,7]
    w147 = np.ascontiguousarray(
        whead.transpose(0, 2, 3, 1).reshape(147, C).T)          # [C,147] m=c*49+dy*7+dx
    w147a = np.ascontiguousarray(w147[:, :128])
    w147b = np.ascontiguousarray(w147[:, 128:])

    imcs = _host_im2col(x)

    # ---- device run ----
    from concourse.bass_utils import run_bass_kernel_spmd
    nc = _build_device_program()
    in_maps = []
    for core in range(8):
        b, h = core // 2, core % 2
        in_maps.append({"imc": imcs[(b, h)], "wbb": w27,
                        "w147a": w147a, "w147b": w147b})
    import time as _time
    _t0 = _time.time()
    res = run_bass_kernel_spmd(nc, in_maps, core_ids=list(range(8)))
    global LAST_EXEC_NS, LAST_DEVICE_S
    LAST_DEVICE_S = _time.time() - _t0
    LAST_EXEC_NS = res.exec_time_ns

    # ---- host: assemble maps ----
    d_map = np.zeros((B, H, W), np.float32)
    ref_map = np.zeros((B, 2, H, W), np.float32)
    for core in range(8):
        b, h = core // 2, core % 2
        maps = _shift_sum(res.results[core])
        sl = slice(h * HALF, (h + 1) * HALF)
        d_map[b, sl] = maps[0, :, 3:3 + W]
        ref_map[b, 0, sl] = maps[1, :, 3:3 + W]
        ref_map[b, 1, sl] = maps[2, :, 3:3 + W]
    ref_map = (MARGIN * np.tanh(ref_map + np.asarray(b_ref, np.float32)[None, :, None, None])).astype(np.float32)
    bd = np.float32(np.asarray(b_score, np.float32)[1] - np.asarray(b_score, np.float32)[0])
    d_map = d_map + bd

    # ---- top-k by softmax-foreground ordering (matches jax softmax+top_k) ----
    dd = d_map.reshape(B, H * W).astype(np.float32)
    pos = dd >= 0
    e = np.exp(np.where(pos, -dd, dd).astype(np.float32)).astype(np.float32)
    fg = np.where(pos, (np.float32(1.0) / (np.float32(1.0) + e)).astype(np.float32),
                  (e / (np.float32(1.0) + e)).astype(np.float32))
    top_idx = np.argsort(-fg, axis=1, kind="stable")[:, :N_DET].astype(np.int32)

    # ---- loc/fourier head values at detections via f-patch matmul ----
    px = (top_idx % W).astype(np.float32)
    py = (top_idx // W).astype(np.float32)
    w22 = np.concatenate([w_loc, w_fourier], 0)       # [22,C,7,7]
    w22f = w22.reshape(22, C * 49)
    b22 = np.concatenate([np.asarray(b_loc, np.float32),
                          np.asarray(b_fourier, np.float32)], 0)
    head22 = np.zeros((B, N_DET, 22), np.float32)
    for b in range(B):
        iy = top_idx[b] // W
        ix = top_idx[b] % W
        h_of = iy // HALF
        srel = (iy - h_of * HALF) // SLAB
        jf = (iy - h_of * HALF) - srel * SLAB + 3     # f-row within slab
        # gather im2col columns for the 7x7 window rows jf-3..jf+3, cols ix..ix+6
        vals = np.zeros((N_DET, C, 49), np.float32)
        for h in range(2):
            m = h_of == h
            if not m.any():
                continue
            imc = imcs[(b, h)].reshape(NSLAB, 27, FROWS, WP)
            sm, jm, xm = srel[m], jf[m], ix[m]
            # columns: (jm + a - 3, xm + bb2) for a,bb2 in 7x7
            a_off = np.arange(7) - 3
            rows = (jm[:, None, None] + a_off[:, None])
            colx = (xm[:, None, None] + np.arange(7))
            patch27 = imc[sm[:, None, None], :, rows, colx]   # [n,7,7,27]
            fwin = np.maximum(
                np.einsum("kc,nabk->nabc", w27, patch27.astype(np.float32),
                          dtype=np.float32) + b_bb[None, None, None, :], 0.0
            ).astype(np.float32)                               # [n,7,7,C]
            vals[m] = fwin.transpose(0, 3, 1, 2).reshape(-1, C, 49)
        head22[b] = vals.reshape(N_DET, C * 49) @ w22f.T + b22[None, :]

    loc = head22[..., 0:2]
    coef = head22[..., 2:22].reshape(B, N_DET, ORDER, 4)
    cx = (px + loc[..., 0]).astype(np.float32)
    cy = (py + loc[..., 1]).astype(np.float32)

    # ---- fourier contour synthesis ----
    t = np.arange(SAMPLES, dtype=np.float32) / np.float32(SAMPLES)
    kk = np.arange(1, ORDER + 1, dtype=np.float32)
    ang = (np.float32(2.0 * np.pi) * kk[:, None] * t[None, :]).astype(np.float32)
    cos_a = np.cos(ang).astype(np.float32)
    sin_a = np.sin(ang).astype(np.float32)
    xs = (np.einsum("bno,os->bns", coef[..., 0], cos_a, dtype=np.float32)
          + np.einsum("bno,os->bns", coef[..., 1], sin_a, dtype=np.float32)
          + cx[..., None]).astype(np.float32)
    ys = (np.einsum("bno,os->bns", coef[..., 2], cos_a, dtype=np.float32)
          + np.einsum("bno,os->bns", coef[..., 3], sin_a, dtype=np.float32)
          + cy[..., None]).astype(np.float32)
    det = np.stack([xs, ys], -1)

    # ---- refinement iterations ----
    ref_flat = ref_map.reshape(B, 2, H * W)
    for _ in range(ITERS):
        deti = np.round(det)
        xc = np.clip(deti[..., 0], 0, W - 1)
        yc = np.clip(deti[..., 1], 0, H - 1)
        lin = (yc.astype(np.int32) * W + xc.astype(np.int32)).reshape(B, N_DET * SAMPLES)
        rx = np.take_along_axis(ref_flat[:, 0], lin, 1).reshape(B, N_DET, SAMPLES)
        ry = np.take_along_axis(ref_flat[:, 1], lin, 1).reshape(B, N_DET, SAMPLES)
        det = np.stack([(xc + rx).astype(np.float32),
                        (yc + ry).astype(np.float32)], -1)
    return det.astype(np.float32)



# revision 9
# speedup vs baseline: 30.1294x; 30.1294x over previous
"""nn_CPN_67740224192953: full conv pipeline on 8 trn2 cores, minimal I/O.

Device (8 cores, 2 per image = half-image each), per 16-row slab:
  - im2col [27, 22*520] built by 9 strided DMAs from a per-core padded
    canvas in DRAM (1.66 MB/core in, vs 19.7 MB host-im2col before)
  - backbone f = relu(w27.T @ imc) fp32 matmuls (f32r breaks top-k rank order)
  - 7x7 head for the 3 needed channels [d=s1-s0, ref_x, ref_y] as two
    7-tap stages: Q[(c,dy)] = sum_dx W_dx.T @ f(. + dx)  (K=64, M=21),
    out[c] = sum_dy S_dy.T @ Q(. + dy*520)               (K=21, M=3)
  - out maps [3, 16, 512] DMA'd out (1.57 MB/core out, vs 630 MB before)
Host: global top/bottom 3-row boundary fix, softmax ordering + top-k,
  loc/fourier head at 512 detections (patch matmul), fourier contour
  synthesis, 4 iterations of refinement gathers (mirrors reference).
The PJRT executor (jit + NEFF) is cached at module level so the second
kernel() call is upload + execute + download only.
"""

import numpy as np

LAST_EXEC_NS = None
LAST_DEVICE_S = None

B, C_IN, H, W = 4, 3, 512, 512
C = 64
ORDER = 5
SAMPLES = 32
N_DET = 512
ITERS = 4
MARGIN = 3.0
K7 = 7
HALF = H // 2         # 256 rows per core
SLAB = 16             # output rows per slab
NSLAB = HALF // SLAB  # 16 slabs
FROWS = SLAB + 6      # f rows per slab (halo 3 top+bottom)
WC = W + 8            # canvas / position-grid width 520
CROWS = HALF + 10     # canvas rows per core 266
LPOS = FROWS * WC     # 11440 f/Q positions per slab
LF = 3 + LPOS + 3     # fpad length
OROWS = SLAB * WC     # 8320 out positions per slab
NCH_F = (LPOS + 511) // 512   # 23 chunks
NCH_O = (OROWS + 511) // 512  # 17 chunks

_RUNNER = None        # (sharded_jit, in_names, out_names, out_avals, n_params)


def _build_device_program():
    import concourse.bacc as bacc
    import concourse.mybir as mybir
    from concourse.tile import TileContext

    nc = bacc.Bacc("TRN2", target_bir_lowering=False, num_devices=8)
    f32 = mybir.dt.float32
    f32r = mybir.dt.float32r
    cv_d = nc.dram_tensor("cv", [3, CROWS * WC], f32, kind="ExternalInput")
    w27_d = nc.dram_tensor("w27", [27, C], f32, kind="ExternalInput")
    wdx_d = nc.dram_tensor("wdx", [C, 147], f32, kind="ExternalInput")
    sdy_d = nc.dram_tensor("sdy", [21, 21], f32, kind="ExternalInput")
    z_d = nc.dram_tensor("z", [C, 128], f32, kind="ExternalInput")
    out_d = nc.dram_tensor("out", [NSLAB * 3, SLAB, W], f32, kind="ExternalOutput")

    with (
        TileContext(nc) as tc,
        tc.tile_pool(name="wpool", bufs=1) as wpool,
        tc.tile_pool(name="sb", bufs=1) as sb,
        tc.tile_pool(name="ps", bufs=2, space="PSUM") as ps,
        tc.tile_pool(name="ps1", bufs=2, space="PSUM") as ps1,
        tc.tile_pool(name="ps2", bufs=2, space="PSUM") as ps2,
    ):
        # weights: DMA in, then re-copy on DVE so every matmul's weight dep
        # is a DVE semaphore (keeps per-matmul sync-wait count at the limit)
        w27_r = wpool.tile([27, C], f32, tag="w27r")
        wdx_r = wpool.tile([C, 147], f32, tag="wdxr")
        sdy_r = wpool.tile([21, 21], f32, tag="sdyr")
        nc.sync.dma_start(out=w27_r[:], in_=w27_d[:, :])
        nc.sync.dma_start(out=wdx_r[:], in_=wdx_d[:, :])
        nc.sync.dma_start(out=sdy_r[:], in_=sdy_d[:, :])
        w27_t = wpool.tile([27, C], f32, tag="w27")
        wdx_t = wpool.tile([C, 147], f32, tag="wdx")
        sdy_t = wpool.tile([21, 21], f32, tag="sdy")
        nc.vector.tensor_copy(w27_t[:], w27_r[:])
        nc.vector.tensor_copy(wdx_t[:], wdx_r[:])
        nc.vector.tensor_copy(sdy_t[:], sdy_r[:])

        # fpad's flat 3-col pads: written once (relu never touches them;
        # their values only reach discarded edge columns of Q)
        fpad_t = sb.tile([C, LF], f32, tag="fpad")
        nc.sync.dma_start(out=fpad_t[:, 0:3], in_=z_d[:, 0:3])
        nc.sync.dma_start(out=fpad_t[:, 3 + LPOS:], in_=z_d[:, 0:3])

        for s in range(NSLAB):
            # im2col: imc[(dy*3+dx)*3+cin, p] = cv[cin, p + (s*16+dy)*520 + dx]
            imc_t = sb.tile([27, LPOS], f32, tag="imc")
            for j in range(9):
                dy, dx = j // 3, j % 3
                off = (s * SLAB + dy) * WC + dx
                nc.sync.dma_start(out=imc_t[3 * j:3 * j + 3, :],
                                  in_=cv_d[:, off:off + LPOS])
            # zero imc's per-row edge cols (q in [0,3) and [515,520)) so the
            # backbone writes f=relu(0)=0 there — the 7x7 zero-padding of f
            # in the reference
            imc3 = imc_t[:].rearrange("p (r w) -> p r w", w=WC)
            nc.sync.dma_start(
                out=imc3[:, :, 0:3],
                in_=z_d[0:27, 0:3 * FROWS].rearrange("p (r w) -> p r w", w=3))
            nc.sync.dma_start(
                out=imc3[:, :, W + 3:WC],
                in_=z_d[0:27, 0:5 * FROWS].rearrange("p (r w) -> p r w", w=5))
            # backbone: f = relu(w27.T @ imc), relu on ACT
            for k in range(NCH_F):
                a, b = k * 512, min((k + 1) * 512, LPOS)
                pbb = ps.tile([C, 512], f32, tag="pbb")
                nc.tensor.matmul(out=pbb[:, :b - a],
                                 lhsT=w27_t[:],
                                 rhs=imc_t[:, a:b],
                                 start=True, stop=True)
                nc.scalar.activation(fpad_t[:, 3 + a:3 + b], pbb[:, :b - a],
                                     mybir.ActivationFunctionType.Relu)
            # stage 1: Q[(c*7+dy), p] = sum_dx wdx[:, dx].T @ fpad[p + dx]
            q_t = sb.tile([21, LPOS], f32, tag="q")
            for k in range(NCH_F):
                a, b = k * 512, min((k + 1) * 512, LPOS)
                pq = ps1.tile([21, 512], f32, tag="pq")
                for dx in range(7):
                    nc.tensor.matmul(out=pq[:, :b - a],
                                     lhsT=wdx_t[:, 21 * dx:21 * dx + 21],
                                     rhs=fpad_t[:, a + dx:b + dx],
                                     start=(dx == 0), stop=(dx == 6))
                nc.vector.tensor_copy(q_t[:, a:b], pq[:, :b - a])
            # stage 2: out[c, p] = sum_dy sdy[:, dy].T @ Q[p + dy*520]
            o_t = sb.tile([3, OROWS], f32, tag="o")
            for k in range(NCH_O):
                a, b = k * 512, min((k + 1) * 512, OROWS)
                po = ps2.tile([3, 512], f32, tag="po")
                for dy in range(7):
                    nc.tensor.matmul(out=po[:, :b - a],
                                     lhsT=sdy_t[:, 3 * dy:3 * dy + 3],
                                     rhs=q_t[:, a + dy * WC:b + dy * WC],
                                     start=(dy == 0), stop=(dy == 6))
                nc.vector.tensor_copy(o_t[:, a:b], po[:, :b - a])
            o3 = o_t[:].rearrange("p (t w) -> p t w", w=WC)
            nc.sync.dma_start(out=out_d[s * 3:(s + 1) * 3, :, :],
                              in_=o3[:, :, 3:3 + W])
    nc.finalize()
    return nc


def _get_runner():
    """Build the program + jitted 8-core PJRT executor once per process."""
    global _RUNNER
    if _RUNNER is not None:
        return _RUNNER
    import jax
    from jax.sharding import Mesh, PartitionSpec
    try:
        from jax.experimental.shard_map import shard_map
    except ImportError:
        from jax.shard_map import shard_map
    import concourse.mybir as mybir
    from concourse.bass2jax import (_bass_exec_p, install_neuronx_cc_hook,
                                    partition_id_tensor)

    install_neuronx_cc_hook()
    nc = _build_device_program()
    partition_name = (nc.partition_id_tensor.name
                      if nc.partition_id_tensor else None)
    in_names, out_names, out_avals, zero_shapes = [], [], [], []
    for alloc in nc.m.functions[0].allocations:
        if not isinstance(alloc, mybir.MemoryLocationSet):
            continue
        name = alloc.memorylocations[0].name
        if alloc.kind == "ExternalInput":
            if name != partition_name:
                in_names.append(name)
        elif alloc.kind == "ExternalOutput":
            out_names.append(name)
            shape = tuple(alloc.tensor_shape)
            dtype = mybir.dt.np(alloc.dtype)
            out_avals.append(jax.core.ShapedArray(shape, dtype))
            zero_shapes.append((shape, dtype))
    n_params = len(in_names)
    n_outs = len(out_avals)
    all_names = in_names + out_names
    if partition_name is not None:
        all_names.append(partition_name)
    donate = tuple(range(n_params, n_params + n_outs))

    def _body(*args):
        operands = list(args)
        if partition_name is not None:
            operands.append(partition_id_tensor())
        outs = _bass_exec_p.bind(
            *operands,
            out_avals=tuple(out_avals),
            in_names=tuple(all_names),
            out_names=tuple(out_names),
            lowering_input_output_aliases=(),
            sim_require_finite=True,
            sim_require_nnan=True,
            nc=nc,
        )
        return tuple(outs)

    devices = jax.devices()[:8]
    mesh = Mesh(np.asarray(devices), ("core",))
    in_specs = (PartitionSpec("core"),) * (n_params + n_outs)
    out_specs = (PartitionSpec("core"),) * n_outs
    sharded = jax.jit(
        shard_map(_body, mesh=mesh, in_specs=in_specs, out_specs=out_specs,
                  check_rep=False),
        donate_argnums=donate, keep_unused=True)
    _RUNNER = (sharded, in_names, out_names, out_avals, zero_shapes)
    return _RUNNER


def _run_device(in_maps):
    """8-core SPMD execute with the cached jit; returns per-core out dicts."""
    sharded, in_names, out_names, out_avals, zero_shapes = _get_runner()
    concat_in = [np.concatenate([m[name] for m in in_maps], axis=0)
                 for name in in_names]
    concat_zeros = [np.zeros((8 * s[0], *s[1:]), d) for s, d in zero_shapes]
    out_arrs = sharded(*concat_in, *concat_zeros)
    return [
        {name: np.asarray(out_arrs[i]).reshape(8, *out_avals[i].shape)[c]
         for i, name in enumerate(out_names)}
        for c in range(8)
    ]


def kernel(x, w_bb, b_bb, w_score, b_score, w_loc, b_loc,
           w_fourier, b_fourier, w_ref, b_ref):
    import time as _time
    x = np.asarray(x, np.float32)
    w_bb = np.asarray(w_bb, np.float32)
    w_score = np.asarray(w_score, np.float32)
    w_loc = np.asarray(w_loc, np.float32)
    w_fourier = np.asarray(w_fourier, np.float32)
    w_ref = np.asarray(w_ref, np.float32)
    b_bb = np.asarray(b_bb, np.float32)

    # ---- weights prep ----
    # w27[(dy*3+dx)*3+cin, cout]
    w27 = np.ascontiguousarray(w_bb.transpose(2, 3, 1, 0).reshape(27, C))
    w_d = (w_score[1] - w_score[0]).astype(np.float32)          # [C,7,7]
    whead = np.stack([w_d, w_ref[0], w_ref[1]], 0)              # [3,C,7,7]
    # wdx[ch, dx*21 + c*7 + dy]
    wdx = np.ascontiguousarray(whead.transpose(1, 3, 0, 2).reshape(C, 147))
    sdy = np.zeros((21, 21), np.float32)
    for c in range(3):
        for dy in range(7):
            sdy[c * 7 + dy, dy * 3 + c] = 1.0
    # ---- canvases: image rows -4..517, cols -4..515, zero-padded ----
    xgfull = np.zeros((B, 3, H + 10, WC), np.float32)
    xgfull[:, :, 4:4 + H, 4:4 + W] = x
    in_maps = []
    for core in range(8):
        b, h = core // 2, core % 2
        cv = np.ascontiguousarray(
            xgfull[b, :, h * HALF:h * HALF + CROWS, :]).reshape(3, CROWS * WC)
        in_maps.append({"cv": cv, "w27": w27, "wdx": wdx, "sdy": sdy,
                        "z": np.zeros((C, 128), np.float32)})

    # ---- device run ----
    _t0 = _time.time()
    res = _run_device(in_maps)
    global LAST_EXEC_NS, LAST_DEVICE_S
    LAST_DEVICE_S = _time.time() - _t0
    LAST_EXEC_NS = None

    # ---- host: assemble maps ----
    d_map = np.zeros((B, H, W), np.float32)
    ref_map = np.zeros((B, 2, H, W), np.float32)
    for core in range(8):
        b, h = core // 2, core % 2
        o = res[core]["out"].reshape(NSLAB, 3, SLAB, W)
        sl = slice(h * HALF, (h + 1) * HALF)
        d_map[b, sl] = o[:, 0].reshape(HALF, W)
        ref_map[b, 0, sl] = o[:, 1].reshape(HALF, W)
        ref_map[b, 1, sl] = o[:, 2].reshape(HALF, W)

    # ---- host fix of global top/bottom 3 rows (f zero-padding there) ----
    swv = np.lib.stride_tricks.sliding_window_view
    xp = np.pad(x, ((0, 0), (0, 0), (1, 1), (1, 1)))
    for b in range(B):
        for top in (True, False):
            rows = np.arange(0, 6) if top else np.arange(H - 6, H)
            # f rows `rows`: conv3x3 at those image rows
            xwin = swv(xp[b, :, rows[0]:rows[-1] + 3, :], (3, 3),
                       axis=(1, 2))                    # [3, 6, 512, 3, 3]
            fv = np.einsum("crXde,ocde->orX", xwin, w_bb,
                           dtype=np.float32) + b_bb[:, None, None]
            fv = np.maximum(fv, 0.0).astype(np.float32)  # [64, 6, 512]
            # zero-padded f block covering out rows Y (3 rows) needs f rows
            # Y-3..Y+3; rows outside [0,H) are zero
            fz = np.zeros((C, 9, W + 6), np.float32)
            if top:
                fz[:, 3:9, 3:3 + W] = fv                 # f rows 0..5
                yo = np.arange(3)
            else:
                fz[:, 0:6, 3:3 + W] = fv                 # f rows H-6..H-1
                yo = np.arange(H - 3, H)
            fwin = swv(fz, (7, 7), axis=(1, 2))          # [64, 3, 512, 7, 7]
            hmap = np.einsum("kYXab,ckab->cYX", fwin, whead, dtype=np.float32)
            d_map[b, yo] = hmap[0]
            ref_map[b, 0, yo] = hmap[1]
            ref_map[b, 1, yo] = hmap[2]

    ref_map = (MARGIN * np.tanh(ref_map + np.asarray(b_ref, np.float32)[None, :, None, None])).astype(np.float32)
    bd = np.float32(np.asarray(b_score, np.float32)[1] - np.asarray(b_score, np.float32)[0])
    d_map = d_map + bd

    # ---- top-k by softmax-foreground ordering (matches jax softmax+top_k) ----
    dd = d_map.reshape(B, H * W).astype(np.float32)
    pos = dd >= 0
    e = np.exp(np.where(pos, -dd, dd).astype(np.float32)).astype(np.float32)
    fg = np.where(pos, (np.float32(1.0) / (np.float32(1.0) + e)).astype(np.float32),
                  (e / (np.float32(1.0) + e)).astype(np.float32))
    top_idx = np.argsort(-fg, axis=1, kind="stable")[:, :N_DET].astype(np.int32)

    # ---- loc/fourier head values at detections via f-patch matmul ----
    px = (top_idx % W).astype(np.float32)
    py = (top_idx // W).astype(np.float32)
    w22 = np.concatenate([w_loc, w_fourier], 0)       # [22,C,7,7]
    w22f = w22.reshape(22, C * 49)
    b22 = np.concatenate([np.asarray(b_loc, np.float32),
                          np.asarray(b_fourier, np.float32)], 0)
    wbb4 = w_bb.transpose(1, 2, 3, 0)                 # [cin,dy,dx,cout]
    head22 = np.zeros((B, N_DET, 22), np.float32)
    for b in range(B):
        iy = top_idx[b] // W
        ix = top_idx[b] % W
        # f window rows iy-3..iy+3, cols ix-3..ix+3; xg windows via swv
        sw = swv(xgfull[b, :, 4 - 4:, :], (3, 3), axis=(1, 2))
        # sw[c, i, j, dy, dx] = xgfull[c, i+dy, j+dx]; f(Y,X) uses rows Y+3+dy
        a_off = np.arange(7)
        rows = iy[:, None, None] + a_off[:, None]
        cols = ix[:, None, None] + a_off[None, :]
        xgwin = sw[:, rows, cols]                     # [3, n, 7, 7, 3, 3]
        fwin = np.einsum("cnabde,cdeo->nabo", xgwin, wbb4,
                         dtype=np.float32) + b_bb[None, None, None, :]
        fwin = np.maximum(fwin, 0.0).astype(np.float32)   # [n,7,7,C]
        vals = fwin.transpose(0, 3, 1, 2).reshape(N_DET, C * 49)
        head22[b] = vals @ w22f.T + b22[None, :]

    loc = head22[..., 0:2]
    coef = head22[..., 2:22].reshape(B, N_DET, ORDER, 4)
    cx = (px + loc[..., 0]).astype(np.float32)
    cy = (py + loc[..., 1]).astype(np.float32)

    # ---- fourier contour synthesis ----
    t = np.arange(SAMPLES, dtype=np.float32) / np.float32(SAMPLES)
    kk = np.arange(1, ORDER + 1, dtype=np.float32)
    ang = (np.float32(2.0 * np.pi) * kk[:, None] * t[None, :]).astype(np.float32)
    cos_a = np.cos(ang).astype(np.float32)
    sin_a = np.sin(ang).astype(np.float32)
    xs = (np.einsum("bno,os->bns", coef[..., 0], cos_a, dtype=np.float32)
          + np.einsum("bno,os->bns", coef[..., 1], sin_a, dtype=np.float32)
          + cx[..., None]).astype(np.float32)
    ys = (np.einsum("bno,os->bns", coef[..., 2], cos_a, dtype=np.float32)
          + np.einsum("bno,os->bns", coef[..., 3], sin_a, dtype=np.float32)
          + cy[..., None]).astype(np.float32)
    det = np.stack([xs, ys], -1)

    # ---- refinement iterations ----
    ref_flat = ref_map.reshape(B, 2, H * W)
    for _ in range(ITERS):
        deti = np.round(det)
        xc = np.clip(deti[..., 0], 0, W - 1)
        yc = np.clip(deti[..., 1], 0, H - 1)
        lin = (yc.astype(np.int32) * W + xc.astype(np.int32)).reshape(B, N_DET * SAMPLES)
        rx = np.take_along_axis(ref_flat[:, 0], lin, 1).reshape(B, N_DET, SAMPLES)
        ry = np.take_along_axis(ref_flat[:, 1], lin, 1).reshape(B, N_DET, SAMPLES)
        det = np.stack([(xc + rx).astype(np.float32),
                        (yc + ry).astype(np.float32)], -1)
    return det.astype(np.float32)


# revision 20
# speedup vs baseline: 54.1229x; 1.7963x over previous
"""nn_CPN_67740224192953: full conv pipeline on 8 trn2 cores, minimal I/O.

Device (8 cores, 2 per image = half-image each), per 16-row slab:
  - im2col [27, 22*520] built by 9 strided DMAs from a per-core padded
    canvas in DRAM (1.66 MB/core in, vs 19.7 MB host-im2col before)
  - backbone f = relu(w27.T @ imc) fp32 matmuls (f32r breaks top-k rank order)
  - 7x7 head for the 3 needed channels [d=s1-s0, ref_x, ref_y] as two
    7-tap stages: Q[(c,dy)] = sum_dx W_dx.T @ f(. + dx)  (K=64, M=21),
    out[c] = sum_dy S_dy.T @ Q(. + dy*520)               (K=21, M=3)
  - out maps [3, 16, 512] DMA'd out (1.57 MB/core out, vs 630 MB before)
Host: global top/bottom 3-row boundary fix, softmax ordering + top-k,
  loc/fourier head at 512 detections (patch matmul), fourier contour
  synthesis, 4 iterations of refinement gathers (mirrors reference).
The PJRT executor (jit + NEFF) is cached at module level so the second
kernel() call is upload + execute + download only.
"""

import numpy as np

LAST_EXEC_NS = None
LAST_DEVICE_S = None

B, C_IN, H, W = 4, 3, 512, 512
C = 64
ORDER = 5
SAMPLES = 32
N_DET = 512
ITERS = 4
MARGIN = 3.0
K7 = 7
HALF = H // 2         # 256 rows per core
SLAB = 16             # output rows per slab
NSLAB = HALF // SLAB  # 16 slabs
FROWS = SLAB + 6      # f rows per slab (halo 3 top+bottom)
WC = W + 8            # canvas / position-grid width 520
CROWS = HALF + 10     # canvas rows per core 266
LPOS = FROWS * WC     # 11440 f/Q positions per slab
LF = 3 + LPOS + 3     # fpad length
OROWS = SLAB * WC     # 8320 out positions per slab
NCH_F = (LPOS + 511) // 512   # 23 chunks
NCH_O = (OROWS + 511) // 512  # 17 chunks

_RUNNER = None        # (sharded_jit, in_names, out_names, out_avals, n_params)


def _build_device_program():
    import concourse.bacc as bacc
    import concourse.mybir as mybir
    from concourse.tile import TileContext

    nc = bacc.Bacc("TRN2", target_bir_lowering=False, num_devices=8)
    f32 = mybir.dt.float32
    f32r = mybir.dt.float32r
    cv_d = nc.dram_tensor("cv", [3, CROWS * WC], f32, kind="ExternalInput")
    w27_d = nc.dram_tensor("w27", [27, C], f32, kind="ExternalInput")
    wdx_d = nc.dram_tensor("wdx", [C, 147], f32, kind="ExternalInput")
    sdy_d = nc.dram_tensor("sdy", [21, 21], f32, kind="ExternalInput")
    z_d = nc.dram_tensor("z", [C, 128], f32, kind="ExternalInput")
    bref_d = nc.dram_tensor("bref", [2, 1], f32, kind="ExternalInput")
    f16 = mybir.dt.float16
    outd_d = nc.dram_tensor("outd", [NSLAB, SLAB, W], f32, kind="ExternalOutput")
    outr_d = nc.dram_tensor("outr", [NSLAB * 2, SLAB, W], f16, kind="ExternalOutput")

    with (
        TileContext(nc) as tc,
        tc.tile_pool(name="wpool", bufs=1) as wpool,
        tc.tile_pool(name="sb", bufs=1) as sb,
        tc.tile_pool(name="ps", bufs=2, space="PSUM") as ps,
        tc.tile_pool(name="ps1", bufs=2, space="PSUM") as ps1,
        tc.tile_pool(name="ps2", bufs=2, space="PSUM") as ps2,
    ):
        # weights: DMA in, then re-copy on DVE so every matmul's weight dep
        # is a DVE semaphore (keeps per-matmul sync-wait count at the limit)
        w27_r = wpool.tile([27, C], f32, tag="w27r")
        wdx_r = wpool.tile([C, 147], f32, tag="wdxr")
        sdy_r = wpool.tile([21, 21], f32, tag="sdyr")
        nc.sync.dma_start(out=w27_r[:], in_=w27_d[:, :])
        nc.sync.dma_start(out=wdx_r[:], in_=wdx_d[:, :])
        nc.sync.dma_start(out=sdy_r[:], in_=sdy_d[:, :])
        bref_r = wpool.tile([2, 1], f32, tag="brefr")
        nc.sync.dma_start(out=bref_r[:], in_=bref_d[:, :])
        w27_t = wpool.tile([27, C], f32, tag="w27")
        wdx_t = wpool.tile([C, 147], f32, tag="wdx")
        sdy_t = wpool.tile([21, 21], f32, tag="sdy")
        bref_t = wpool.tile([2, 1], f32, tag="bref")
        nc.vector.tensor_copy(w27_t[:], w27_r[:])
        nc.vector.tensor_copy(wdx_t[:], wdx_r[:])
        nc.vector.tensor_copy(sdy_t[:], sdy_r[:])
        nc.vector.tensor_copy(bref_t[:], bref_r[:])

        # fpad's flat 3-col pads: written once (relu never touches them;
        # their values only reach discarded edge columns of Q)
        fpad_t = sb.tile([C, LF], f32, tag="fpad")
        nc.sync.dma_start(out=fpad_t[:, 0:3], in_=z_d[:, 0:3])
        nc.sync.dma_start(out=fpad_t[:, 3 + LPOS:], in_=z_d[:, 0:3])

        for s in range(NSLAB):
            # im2col: imc[(dy*3+dx)*3+cin, p] = cv[cin, p + (s*16+dy)*520 + dx]
            imc_t = sb.tile([27, LPOS], f32, tag="imc")
            for j in range(9):
                dy, dx = j // 3, j % 3
                off = (s * SLAB + dy) * WC + dx
                nc.sync.dma_start(out=imc_t[3 * j:3 * j + 3, :],
                                  in_=cv_d[:, off:off + LPOS])
            # zero imc's per-row edge cols (q in [0,3) and [515,520)) so the
            # backbone writes f=relu(0)=0 there — the 7x7 zero-padding of f
            # in the reference
            imc3 = imc_t[:].rearrange("p (r w) -> p r w", w=WC)
            nc.sync.dma_start(
                out=imc3[:, :, 0:3],
                in_=z_d[0:27, 0:3 * FROWS].rearrange("p (r w) -> p r w", w=3))
            nc.sync.dma_start(
                out=imc3[:, :, W + 3:WC],
                in_=z_d[0:27, 0:5 * FROWS].rearrange("p (r w) -> p r w", w=5))
            # backbone: f = relu(w27.T @ imc), relu on ACT
            for k in range(NCH_F):
                a, b = k * 512, min((k + 1) * 512, LPOS)
                pbb = ps.tile([C, 512], f32, tag="pbb")
                nc.tensor.matmul(out=pbb[:, :b - a],
                                 lhsT=w27_t[:],
                                 rhs=imc_t[:, a:b],
                                 start=True, stop=True)
                nc.scalar.activation(fpad_t[:, 3 + a:3 + b], pbb[:, :b - a],
                                     mybir.ActivationFunctionType.Relu)
            # stage 1: Q[(c*7+dy), p] = sum_dx wdx[:, dx].T @ fpad[p + dx]
            q_t = sb.tile([21, LPOS], f32, tag="q")
            for k in range(NCH_F):
                a, b = k * 512, min((k + 1) * 512, LPOS)
                pq = ps1.tile([21, 512], f32, tag="pq")
                for dx in range(7):
                    nc.tensor.matmul(out=pq[:, :b - a],
                                     lhsT=wdx_t[:, 21 * dx:21 * dx + 21],
                                     rhs=fpad_t[:, a + dx:b + dx],
                                     start=(dx == 0), stop=(dx == 6))
                nc.vector.tensor_copy(q_t[:, a:b], pq[:, :b - a])
            # stage 2: out[c, p] = sum_dy sdy[:, dy].T @ Q[p + dy*520];
            # d stays f32, the two ref channels leave as fp16 tanh(conv+b_ref)
            o_t = sb.tile([3, OROWS], f32, tag="o")
            orf_t = sb.tile([2, OROWS], f16, tag="orf")
            for k in range(NCH_O):
                a, b = k * 512, min((k + 1) * 512, OROWS)
                po = ps2.tile([3, 512], f32, tag="po")
                for dy in range(7):
                    nc.tensor.matmul(out=po[:, :b - a],
                                     lhsT=sdy_t[:, 3 * dy:3 * dy + 3],
                                     rhs=q_t[:, a + dy * WC:b + dy * WC],
                                     start=(dy == 0), stop=(dy == 6))
                nc.vector.tensor_copy(o_t[:, a:b], po[:, :b - a])
                nc.scalar.activation(orf_t[:, a:b], po[0:2, :b - a],
                                     mybir.ActivationFunctionType.Tanh,
                                     bias=bref_t[:])
            od3 = o_t[2:3, :].rearrange("p (t w) -> p t w", w=WC)
            orf3 = orf_t[:].rearrange("p (t w) -> p t w", w=WC)
            nc.sync.dma_start(out=outd_d[s:s + 1, :, :], in_=od3[:, :, 3:3 + W])
            nc.sync.dma_start(out=outr_d[s * 2:(s + 1) * 2, :, :],
                              in_=orf3[:, :, 3:3 + W])
    nc.finalize()
    return nc


def _get_runner():
    """Build the program + jitted 8-core PJRT executor once per process."""
    global _RUNNER
    if _RUNNER is not None:
        return _RUNNER
    import jax
    from jax.sharding import Mesh, PartitionSpec
    try:
        from jax.experimental.shard_map import shard_map
    except ImportError:
        from jax.shard_map import shard_map
    import concourse.mybir as mybir
    from concourse.bass2jax import (_bass_exec_p, install_neuronx_cc_hook,
                                    partition_id_tensor)

    install_neuronx_cc_hook()
    nc = _build_device_program()
    partition_name = (nc.partition_id_tensor.name
                      if nc.partition_id_tensor else None)
    in_names, out_names, out_avals, zero_shapes = [], [], [], []
    for alloc in nc.m.functions[0].allocations:
        if not isinstance(alloc, mybir.MemoryLocationSet):
            continue
        name = alloc.memorylocations[0].name
        if alloc.kind == "ExternalInput":
            if name != partition_name:
                in_names.append(name)
        elif alloc.kind == "ExternalOutput":
            out_names.append(name)
            shape = tuple(alloc.tensor_shape)
            dtype = mybir.dt.np(alloc.dtype)
            out_avals.append(jax.core.ShapedArray(shape, dtype))
            zero_shapes.append((shape, dtype))
    n_params = len(in_names)
    n_outs = len(out_avals)
    all_names = in_names + out_names
    if partition_name is not None:
        all_names.append(partition_name)
    donate = tuple(range(n_params, n_params + n_outs))

    def _body(*args):
        operands = list(args)
        if partition_name is not None:
            operands.append(partition_id_tensor())
        outs = _bass_exec_p.bind(
            *operands,
            out_avals=tuple(out_avals),
            in_names=tuple(all_names),
            out_names=tuple(out_names),
            lowering_input_output_aliases=(),
            sim_require_finite=True,
            sim_require_nnan=True,
            nc=nc,
        )
        return tuple(outs)

    devices = jax.devices()[:8]
    mesh = Mesh(np.asarray(devices), ("core",))
    in_specs = (PartitionSpec("core"),) * (n_params + n_outs)
    out_specs = (PartitionSpec("core"),) * n_outs
    sharded = jax.jit(
        shard_map(_body, mesh=mesh, in_specs=in_specs, out_specs=out_specs,
                  check_rep=False),
        donate_argnums=donate, keep_unused=True)
    # donated output buffers created on-device (no 12.6 MB H2D of zeros)
    import jax.numpy as jnp
    from jax.sharding import NamedSharding
    zsh = tuple(NamedSharding(mesh, PartitionSpec("core")) for _ in zero_shapes)

    def _zmake():
        return tuple(jnp.zeros((8 * s[0], *s[1:]), d) for s, d in zero_shapes)
    zeros_jit = jax.jit(_zmake, out_shardings=zsh)
    _RUNNER = (sharded, in_names, out_names, out_avals, zero_shapes, zeros_jit)
    return _RUNNER


def _run_device(in_maps):
    """8-core SPMD execute with the cached jit; returns per-core out dicts."""
    sharded, in_names, out_names, out_avals, zero_shapes, zeros_jit = _get_runner()
    concat_zeros = zeros_jit()     # async on-device; overlaps input upload
    concat_in = [np.concatenate([m[name] for m in in_maps], axis=0)
                 for name in in_names]
    out_arrs = sharded(*concat_in, *concat_zeros)
    return [
        {name: np.asarray(out_arrs[i]).reshape(8, *out_avals[i].shape)[c]
         for i, name in enumerate(out_names)}
        for c in range(8)
    ]


def kernel(x, w_bb, b_bb, w_score, b_score, w_loc, b_loc,
           w_fourier, b_fourier, w_ref, b_ref):
    import time as _time
    x = np.asarray(x, np.float32)
    w_bb = np.asarray(w_bb, np.float32)
    w_score = np.asarray(w_score, np.float32)
    w_loc = np.asarray(w_loc, np.float32)
    w_fourier = np.asarray(w_fourier, np.float32)
    w_ref = np.asarray(w_ref, np.float32)
    b_bb = np.asarray(b_bb, np.float32)

    # ---- weights prep ----
    # w27[(dy*3+dx)*3+cin, cout]
    w27 = np.ascontiguousarray(w_bb.transpose(2, 3, 1, 0).reshape(27, C))
    w_d = (w_score[1] - w_score[0]).astype(np.float32)          # [C,7,7]
    whead = np.stack([w_d, w_ref[0], w_ref[1]], 0)              # [3,C,7,7]
    # wdx[ch, dx*21 + c*7 + dy]
    wdx = np.ascontiguousarray(whead.transpose(1, 3, 0, 2).reshape(C, 147))
    # stage-2 selection; output channel order [ref_x, ref_y, d] so the ACT
    # tanh reads PSUM partitions 0:2 (32-aligned base required)
    perm = {0: 2, 1: 0, 2: 1}
    sdy = np.zeros((21, 21), np.float32)
    for c in range(3):
        for dy in range(7):
            sdy[c * 7 + dy, dy * 3 + perm[c]] = 1.0
    # ---- canvases: image rows -4..517, cols -4..515, zero-padded ----
    xgfull = np.zeros((B, 3, H + 10, WC), np.float32)
    xgfull[:, :, 4:4 + H, 4:4 + W] = x
    in_maps = []
    for core in range(8):
        b, h = core // 2, core % 2
        cv = np.ascontiguousarray(
            xgfull[b, :, h * HALF:h * HALF + CROWS, :]).reshape(3, CROWS * WC)
        in_maps.append({"cv": cv, "w27": w27, "wdx": wdx, "sdy": sdy,
                        "z": np.zeros((C, 128), np.float32),
                        "bref": np.asarray(b_ref, np.float32).reshape(2, 1)})

    # ---- device run ----
    _t0 = _time.time()
    res = _run_device(in_maps)
    global LAST_EXEC_NS, LAST_DEVICE_S
    LAST_DEVICE_S = _time.time() - _t0
    LAST_EXEC_NS = None

    # ---- host: assemble maps ----
    d_map = np.zeros((B, H, W), np.float32)
    ref_map = np.zeros((B, 2, H, W), np.float32)  # MARGIN*tanh(conv+b_ref)
    for core in range(8):
        b, h = core // 2, core % 2
        sl = slice(h * HALF, (h + 1) * HALF)
        d_map[b, sl] = res[core]["outd"].reshape(HALF, W)
        orr = res[core]["outr"].astype(np.float32).reshape(NSLAB, 2, SLAB, W)
        ref_map[b, 0, sl] = MARGIN * orr[:, 0].reshape(HALF, W)
        ref_map[b, 1, sl] = MARGIN * orr[:, 1].reshape(HALF, W)

    # ---- host fix of global top/bottom 3 rows (f zero-padding there) ----
    swv = np.lib.stride_tricks.sliding_window_view
    xp = np.pad(x, ((0, 0), (0, 0), (1, 1), (1, 1)))
    for b in range(B):
        for top in (True, False):
            rows = np.arange(0, 6) if top else np.arange(H - 6, H)
            # f rows `rows`: conv3x3 at those image rows
            xwin = swv(xp[b, :, rows[0]:rows[-1] + 3, :], (3, 3),
                       axis=(1, 2))                    # [3, 6, 512, 3, 3]
            fv = np.einsum("crXde,ocde->orX", xwin, w_bb,
                           dtype=np.float32) + b_bb[:, None, None]
            fv = np.maximum(fv, 0.0).astype(np.float32)  # [64, 6, 512]
            # zero-padded f block covering out rows Y (3 rows) needs f rows
            # Y-3..Y+3; rows outside [0,H) are zero
            fz = np.zeros((C, 9, W + 6), np.float32)
            if top:
                fz[:, 3:9, 3:3 + W] = fv                 # f rows 0..5
                yo = np.arange(3)
            else:
                fz[:, 0:6, 3:3 + W] = fv                 # f rows H-6..H-1
                yo = np.arange(H - 3, H)
            fwin = swv(fz, (7, 7), axis=(1, 2))          # [64, 3, 512, 7, 7]
            hmap = np.einsum("kYXab,ckab->cYX", fwin, whead, dtype=np.float32)
            d_map[b, yo] = hmap[0]
            br = np.asarray(b_ref, np.float32)
            ref_map[b, 0, yo] = MARGIN * np.tanh(hmap[1] + br[0])
            ref_map[b, 1, yo] = MARGIN * np.tanh(hmap[2] + br[1])

    bd = np.float32(np.asarray(b_score, np.float32)[1] - np.asarray(b_score, np.float32)[0])
    d_map = d_map + bd

    # ---- top-k by softmax-foreground ordering (matches jax softmax+top_k) ----
    dd = d_map.reshape(B, H * W).astype(np.float32)
    pos = dd >= 0
    e = np.exp(np.where(pos, -dd, dd).astype(np.float32)).astype(np.float32)
    fg = np.where(pos, (np.float32(1.0) / (np.float32(1.0) + e)).astype(np.float32),
                  (e / (np.float32(1.0) + e)).astype(np.float32))
    top_idx = np.argsort(-fg, axis=1, kind="stable")[:, :N_DET].astype(np.int32)

    # ---- loc/fourier head values at detections via f-patch matmul ----
    px = (top_idx % W).astype(np.float32)
    py = (top_idx // W).astype(np.float32)
    w22 = np.concatenate([w_loc, w_fourier], 0)       # [22,C,7,7]
    w22f = w22.reshape(22, C * 49)
    b22 = np.concatenate([np.asarray(b_loc, np.float32),
                          np.asarray(b_fourier, np.float32)], 0)
    wbb4 = w_bb.transpose(1, 2, 3, 0)                 # [cin,dy,dx,cout]
    head22 = np.zeros((B, N_DET, 22), np.float32)
    for b in range(B):
        iy = top_idx[b] // W
        ix = top_idx[b] % W
        # f window rows iy-3..iy+3, cols ix-3..ix+3; xg windows via swv
        sw = swv(xgfull[b, :, 4 - 4:, :], (3, 3), axis=(1, 2))
        # sw[c, i, j, dy, dx] = xgfull[c, i+dy, j+dx]; f(Y,X) uses rows Y+3+dy
        a_off = np.arange(7)
        rows = iy[:, None, None] + a_off[:, None]
        cols = ix[:, None, None] + a_off[None, :]
        xgwin = sw[:, rows, cols]                     # [3, n, 7, 7, 3, 3]
        fwin = np.einsum("cnabde,cdeo->nabo", xgwin, wbb4,
                         dtype=np.float32) + b_bb[None, None, None, :]
        fwin = np.maximum(fwin, 0.0).astype(np.float32)   # [n,7,7,C]
        vals = fwin.transpose(0, 3, 1, 2).reshape(N_DET, C * 49)
        head22[b] = vals @ w22f.T + b22[None, :]

    loc = head22[..., 0:2]
    coef = head22[..., 2:22].reshape(B, N_DET, ORDER, 4)
    cx = (px + loc[..., 0]).astype(np.float32)
    cy = (py + loc[..., 1]).astype(np.float32)

    # ---- fourier contour synthesis ----
    t = np.arange(SAMPLES, dtype=np.float32) / np.float32(SAMPLES)
    kk = np.arange(1, ORDER + 1, dtype=np.float32)
    ang = (np.float32(2.0 * np.pi) * kk[:, None] * t[None, :]).astype(np.float32)
    cos_a = np.cos(ang).astype(np.float32)
    sin_a = np.sin(ang).astype(np.float32)
    xs = (np.einsum("bno,os->bns", coef[..., 0], cos_a, dtype=np.float32)
          + np.einsum("bno,os->bns", coef[..., 1], sin_a, dtype=np.float32)
          + cx[..., None]).astype(np.float32)
    ys = (np.einsum("bno,os->bns", coef[..., 2], cos_a, dtype=np.float32)
          + np.einsum("bno,os->bns", coef[..., 3], sin_a, dtype=np.float32)
          + cy[..., None]).astype(np.float32)
    det = np.stack([xs, ys], -1)

    # ---- refinement iterations ----
    ref_flat = ref_map.reshape(B, 2, H * W)
    for _ in range(ITERS):
        deti = np.round(det)
        xc = np.clip(deti[..., 0], 0, W - 1)
        yc = np.clip(deti[..., 1], 0, H - 1)
        lin = (yc.astype(np.int32) * W + xc.astype(np.int32)).reshape(B, N_DET * SAMPLES)
        rx = np.take_along_axis(ref_flat[:, 0], lin, 1).reshape(B, N_DET, SAMPLES)
        ry = np.take_along_axis(ref_flat[:, 1], lin, 1).reshape(B, N_DET, SAMPLES)
        det = np.stack([(xc + rx).astype(np.float32),
                        (yc + ry).astype(np.float32)], -1)
    return det.astype(np.float32)
